# revision 20
# baseline (speedup 1.0000x reference)
"""Conv2d 3x3 (stride 1, pad 1) forward on 8 Trainium2 NeuronCores.

Problem: x (32,32,128,128) NCHW fp32, kernel (3,3,32,64) whose raw buffer is
reinterpreted as w_eff (C=32, kh, kw, O=64).  out (32,64,128,128) fp32.

Strategy (data-parallel over batch, 4 images per core):
  - X tile [96 partitions = kh*32+c, 16386] fp32r, image rows stored
    CONTIGUOUSLY (cell(h,w) = 1 + h*128 + w).  Partition kh*32+c holds the
    kh-shifted image x[c, h+kh-1, :] so one K=96 matmul contracts (c,kh).
    Contiguous layout -> 1 DMA descriptor per partition (line rate).
  - Center block (kh=1) from HBM; kh=0 / kh=2 blocks are +-1-row-shifted
    copies, spread across DVE / ACT / SBUF-SBUF DMA / HBM re-read so no
    single resource becomes the bottleneck.
  - Conv as matmul: 3 accumulating matmuls per 512-px tile (kw taps via
    free-dim offsets kw-1).  Reading offset -1/+1 wraps at row edges, so
    columns w=0 (kw=0 tap) and w=127 (kw=2 tap) pick up a spurious term.
    Two N=64 "correction" matmuls per half-image compute those spurious
    terms with negated weights; a per-psum-tile DVE op adds them
    (subtracting the garbage) before eviction.
  - float32r (1 cycle/row, ~1e-4 rel err).  Evictions [64,1024] alternate
    DVE / ACT into a [128,4096] staging chunk (partition = g*64+o), one
    2 MiB DMA out per half-image.
"""

import numpy as np

N_CORES = 8
N_BATCH = 32
IMGS = N_BATCH // N_CORES  # images per core
C = 32
O = 64
H = 128
W = 128
XCELLS = H * W + 2  # headroom cell 0, data 1..16385
TILE = 512  # pixels per matmul (one psum bank)
CHUNK_PX = 4096  # free size of one staging chunk (per g-group)

_CACHE = {}


def _build():
    import concourse.bacc as bacc
    import concourse.tile as tile
    from concourse import mybir

    DT = mybir.dt.float32r
    F32 = mybir.dt.float32

    nc = bacc.Bacc(None, target_bir_lowering=False)
    x_dram = nc.dram_tensor("x", [IMGS, C, H, W], DT, kind="ExternalInput")
    # 5 slices: W0, W1, W2, -W0, -W2
    w_dram = nc.dram_tensor("w", [5, 96, O], DT, kind="ExternalInput")
    out_dram = nc.dram_tensor("out", [IMGS, O, H, W], F32, kind="ExternalOutput")

    # [n, half, g, o, j]: DMA dest iterates (g, o, j) == stage partition
    # order (g*64+o) then free j.  dma_start only checks total size.
    out_v = out_dram.rearrange("n o h w -> n o (h w)").rearrange(
        "n o (c g j) -> n c g o j", c=2, g=2, j=CHUNK_PX
    )

    with tile.TileContext(nc) as tc:
        with (
            tc.tile_pool(name="xbuf", bufs=1) as xpool,
            tc.tile_pool(name="wpool", bufs=1) as wpool,
            tc.tile_pool(name="stage", bufs=3) as spool,
            tc.tile_pool(name="psum", bufs=3, space="PSUM") as ppool,
        ):
            wt = wpool.tile([96, 5 * O], DT, tag="w")
            nc.sync.dma_start(wt[:], w_dram.rearrange("k p o -> p k o"))

            xbufs = [
                xpool.tile([96, XCELLS], DT, tag=f"x{i}", name=f"xb{i}")
                for i in range(2)
            ]
            # One-time zeroing (cells never rewritten per-image):
            #  - center headroom cells 0 and 16385
            #  - kh0 row 0 (cells 0..129)
            #  - kh2 row 127 + headroom (cells 16257..16386)
            for xb in xbufs:
                nc.vector.memset(xb[32:64, 0:1].bitcast(F32), 0.0)
                nc.vector.memset(xb[32:64, XCELLS - 1 : XCELLS].bitcast(F32), 0.0)
                nc.vector.memset(xb[0:32, 0 : W + 1].bitcast(F32), 0.0)
                nc.vector.memset(xb[64:96, XCELLS - W - 1 : XCELLS].bitcast(F32), 0.0)

            for n in range(IMGS):
                xb = xbufs[n % 2]
                # center block (kh=1): partitions 32..64, cells 1..16385
                nc.gpsimd.dma_start(xb[32:64, 1 : 1 + H * W], x_dram[n])
                # kh=0 block: row h <- x[h-1]: cells 129.. <- center 1..16257
                nc.vector.tensor_copy(
                    xb[0:32, W + 1 : XCELLS - 1], xb[32:64, 1 : XCELLS - 1 - W]
                )
                # kh=2 block: row h <- x[h+1]: cells 1..16257 <- center 129..
                if n % 2 == 0:
                    nc.scalar.copy(
                        xb[64:96, 1 : XCELLS - 1 - W], xb[32:64, W + 1 : XCELLS - 1]
                    )
                else:
                    nc.gpsimd.dma_start(
                        xb[64:96, 1 : XCELLS - 1 - W],
                        x_dram[n, :, 1:H, :],
                    )

                for half in range(2):
                    stage = spool.tile([128, CHUNK_PX], F32, tag="stage")
                    corr = ppool.tile([O, 2 * O], F32, tag="corr", bufs=2)
                    corr_sb = spool.tile(
                        [O, 2 * O], F32, tag="corr_sb", bufs=2, name="corr_sb"
                    )
                    hbase = half * 64
                    # E0[o,h]: garbage term added at w=0 by the kw=0 tap
                    nc.tensor.matmul(
                        corr[:, 0:O],
                        wt[:, 3 * O : 4 * O],
                        xb[:, hbase * W : hbase * W + 63 * W + 1 : W],
                        start=True,
                        stop=True,
                    )
                    # E2[o,h]: garbage term added at w=127 by the kw=2 tap
                    nc.tensor.matmul(
                        corr[:, O : 2 * O],
                        wt[:, 4 * O : 5 * O],
                        xb[:, hbase * W + W + 1 : hbase * W + W + 1 + 63 * W + 1 : W],
                        start=True,
                        stop=True,
                    )
                    nc.vector.tensor_copy(corr_sb[:], corr[:])
                    for i2 in range(8):  # 8 psum tiles per half-image
                        ps = ppool.tile([O, 2 * TILE], F32, tag="ps")
                        for hp in range(2):
                            t = half * 16 + i2 * 2 + hp
                            cell0 = t * 4 * W  # row h0 = 4t
                            for kw in range(3):
                                nc.tensor.matmul(
                                    ps[:, hp * TILE : (hp + 1) * TILE],
                                    wt[:, kw * O : (kw + 1) * O],
                                    xb[:, cell0 + kw : cell0 + kw + TILE],
                                    start=(kw == 0),
                                    stop=(kw == 2),
                                )
                        # add the (negated) spurious terms at w=0 / w=127
                        # positions: {r*128} u {r*128+127}, r = row in tile
                        edge = ps[:].rearrange("p (r w) -> p r w", w=W)[
                            :, :, 0 : W : W - 1
                        ]
                        cslice = corr_sb[:].rearrange("p (j h) -> p h j", j=2)[
                            :, i2 * 8 : i2 * 8 + 8, :
                        ]
                        nc.vector.tensor_add(edge, edge, cslice)
                        g = i2 // 4
                        j0 = (i2 % 4) * 2 * TILE
                        dst = stage[g * O : (g + 1) * O, j0 : j0 + 2 * TILE]
                        if i2 % 3 == 0:
                            nc.vector.tensor_copy(dst, ps[:])
                        else:
                            nc.scalar.copy(dst, ps[:])
                    nc.gpsimd.dma_start(out_v[n, half], stage[:])

    nc.compile()
    return nc


def _get_nc():
    if "nc" not in _CACHE:
        _CACHE["nc"] = _build()
    return _CACHE["nc"]


def _prep_weights(kernel: np.ndarray) -> np.ndarray:
    # reference reinterprets the raw (3,3,32,64) buffer as (C=32, kh, kw, O)
    w_eff = np.ascontiguousarray(kernel, dtype=np.float32).reshape(C, 3, 3, O)
    # wk[kw, kh*32+c, o] = w_eff[c, kh, kw, o]
    wk = np.ascontiguousarray(w_eff.transpose(2, 1, 0, 3).reshape(3, 96, O))
    return np.concatenate([wk, -wk[0:1], -wk[2:3]], axis=0)


def run(x: np.ndarray, kernel: np.ndarray, trace: bool = False):
    from concourse.bass_utils import run_bass_kernel_spmd

    if trace:
        try:
            from antenv.axon_hooks import (
                get_axon_ntff_profile_hook,
                set_axon_ntff_profile_hook,
            )
            from trn_agent_boot.trn_boot import _ntff_profile_via_ctypes

            if get_axon_ntff_profile_hook() is None:
                set_axon_ntff_profile_hook(
                    _ntff_profile_via_ctypes("/opt/axon/libaxon_pjrt.so")
                )
        except Exception:
            trace = False

    nc = _get_nc()
    x = np.ascontiguousarray(x, dtype=np.float32)
    wp = _prep_weights(kernel)
    in_maps = [
        {"x": x[i * IMGS : (i + 1) * IMGS], "w": wp} for i in range(N_CORES)
    ]
    res = run_bass_kernel_spmd(
        nc, in_maps, core_ids=list(range(N_CORES)), trace=trace
    )
    out = np.concatenate([res.results[i]["out"] for i in range(N_CORES)], axis=0)
    return out, res


def kernel(x: np.ndarray, kernel: np.ndarray) -> np.ndarray:
    out, _ = run(x, kernel)
    return out


if __name__ == "__main__":
    xs = np.random.randn(N_BATCH, C, H, W).astype(np.float32)
    ks = np.random.randn(3, 3, C, O).astype(np.float32)
    out = kernel(xs, ks)
    print(out.shape, out.dtype)


# revision 23
# speedup vs baseline: 1.6483x; 1.6483x over previous
"""Conv2d 3x3 (stride 1, pad 1) forward on 8 Trainium2 NeuronCores.

Problem: x (32,32,128,128) NCHW fp32, kernel (3,3,32,64) whose raw buffer is
reinterpreted as w_eff (C=32, kh, kw, O=64).  out (32,64,128,128) fp32.

Strategy (data-parallel over batch, 4 images per core):
  - X tile [96 partitions = kh*32+c, 16386] fp32r, image rows stored
    CONTIGUOUSLY (cell(h,w) = 1 + h*128 + w).  Partition kh*32+c holds the
    kh-shifted image x[c, h+kh-1, :] so one K=96 matmul contracts (c,kh).
    Contiguous layout -> 1 DMA descriptor per partition (line rate).
  - Center block (kh=1) from HBM; kh=0 / kh=2 blocks are +-1-row-shifted
    copies, spread across DVE / ACT / SBUF-SBUF DMA / HBM re-read so no
    single resource becomes the bottleneck.
  - Conv as matmul: 3 accumulating matmuls per 512-px tile (kw taps via
    free-dim offsets kw-1).  Reading offset -1/+1 wraps at row edges, so
    columns w=0 (kw=0 tap) and w=127 (kw=2 tap) pick up a spurious term.
    Two N=64 "correction" matmuls per half-image compute those spurious
    terms with negated weights; a per-psum-tile DVE op adds them
    (subtracting the garbage) before eviction.
  - float32r (1 cycle/row, ~1e-4 rel err).  Evictions [64,1024] alternate
    DVE / ACT into a [128,4096] staging chunk (partition = g*64+o), one
    2 MiB DMA out per half-image.
"""

import numpy as np

N_CORES = 8
N_BATCH = 32
IMGS = N_BATCH // N_CORES  # images per core
C = 32
O = 64
H = 128
W = 128
XCELLS = H * W + 2  # headroom cell 0, data 1..16385
TILE = 512  # pixels per matmul (one psum bank)
CHUNK_PX = 4096  # free size of one staging chunk (per g-group)

_CACHE = {}


def _build():
    import concourse.bacc as bacc
    import concourse.tile as tile
    from concourse import mybir

    DT = mybir.dt.float32r
    F32 = mybir.dt.float32

    nc = bacc.Bacc(None, target_bir_lowering=False)
    x_dram = nc.dram_tensor("x", [IMGS, C, H, W], DT, kind="ExternalInput")
    # 5 slices: W0, W1, W2, -W0, -W2
    w_dram = nc.dram_tensor("w", [5, 96, O], DT, kind="ExternalInput")
    out_dram = nc.dram_tensor("out", [IMGS, O, H, W], F32, kind="ExternalOutput")

    # [n, half, g, o, j]: per-(half,g) stores are clean 2D [64, 4096] APs so
    # walrus lowers them to static model-queue descriptors (16-engine spread).
    out_v = out_dram.rearrange("n o h w -> n o (h w)").rearrange(
        "n o (c g j) -> n c g o j", c=2, g=2, j=CHUNK_PX
    )

    with tile.TileContext(nc) as tc:
        with (
            tc.tile_pool(name="xbuf", bufs=1) as xpool,
            tc.tile_pool(name="wpool", bufs=1) as wpool,
            tc.tile_pool(name="stage", bufs=3) as spool,
            tc.tile_pool(name="psum", bufs=3, space="PSUM") as ppool,
        ):
            wt = wpool.tile([96, 5 * O], DT, tag="w")
            nc.sync.dma_start(wt[:], w_dram.rearrange("k p o -> p k o"))

            xbufs = [
                xpool.tile([96, XCELLS], DT, tag=f"x{i}", name=f"xb{i}")
                for i in range(2)
            ]
            # One-time zeroing (cells never rewritten per-image):
            #  - center headroom cells 0 and 16385
            #  - kh0 row 0 (cells 0..129)
            #  - kh2 row 127 + headroom (cells 16257..16386)
            for xb in xbufs:
                nc.vector.memset(xb[32:64, 0:1].bitcast(F32), 0.0)
                nc.vector.memset(xb[32:64, XCELLS - 1 : XCELLS].bitcast(F32), 0.0)
                nc.vector.memset(xb[0:32, 0 : W + 1].bitcast(F32), 0.0)
                nc.vector.memset(xb[64:96, XCELLS - W - 1 : XCELLS].bitcast(F32), 0.0)

            def load_image(n):
                xb = xbufs[n % 2]
                # center block (kh=1): partitions 32..64, cells 1..16385
                nc.sync.dma_start(xb[32:64, 1 : 1 + H * W], x_dram[n])
                # kh=0 block: row h <- x[h-1]: cells 129.. <- center 1..16257
                nc.vector.tensor_copy(
                    xb[0:32, W + 1 : XCELLS - 1], xb[32:64, 1 : XCELLS - 1 - W]
                )
                # kh=2 block: row h <- x[h+1]: cells 1..16257 <- center 129..
                if n % 2 == 0:
                    nc.scalar.copy(
                        xb[64:96, 1 : XCELLS - 1 - W], xb[32:64, W + 1 : XCELLS - 1]
                    )
                else:
                    nc.sync.dma_start(
                        xb[64:96, 1 : XCELLS - 1 - W],
                        x_dram[n, :, 1:H, :],
                    )

            load_image(0)
            load_image(1)
            for n in range(IMGS):
                xb = xbufs[n % 2]
                for half in range(2):
                    stage = spool.tile([128, CHUNK_PX], F32, tag="stage")
                    corr = ppool.tile([O, 2 * O], F32, tag="corr", bufs=2)
                    corr_sb = spool.tile(
                        [O, 2 * O], F32, tag="corr_sb", bufs=2, name="corr_sb"
                    )
                    hbase = half * 64
                    # E0[o,h]: garbage term added at w=0 by the kw=0 tap
                    nc.tensor.matmul(
                        corr[:, 0:O],
                        wt[:, 3 * O : 4 * O],
                        xb[:, hbase * W : hbase * W + 63 * W + 1 : W],
                        start=True,
                        stop=True,
                    )
                    # E2[o,h]: garbage term added at w=127 by the kw=2 tap
                    nc.tensor.matmul(
                        corr[:, O : 2 * O],
                        wt[:, 4 * O : 5 * O],
                        xb[:, hbase * W + W + 1 : hbase * W + W + 1 + 63 * W + 1 : W],
                        start=True,
                        stop=True,
                    )
                    nc.vector.tensor_copy(corr_sb[:], corr[:])
                    for i2 in range(8):  # 8 psum tiles per half-image
                        ps = ppool.tile([O, 2 * TILE], F32, tag="ps")
                        for hp in range(2):
                            t = half * 16 + i2 * 2 + hp
                            cell0 = t * 4 * W  # row h0 = 4t
                            for kw in range(3):
                                nc.tensor.matmul(
                                    ps[:, hp * TILE : (hp + 1) * TILE],
                                    wt[:, kw * O : (kw + 1) * O],
                                    xb[:, cell0 + kw : cell0 + kw + TILE],
                                    start=(kw == 0),
                                    stop=(kw == 2),
                                )
                        # add the (negated) spurious terms at w=0 / w=127
                        # positions: {r*128} u {r*128+127}, r = row in tile
                        edge = ps[:].rearrange("p (r w) -> p r w", w=W)[
                            :, :, 0 : W : W - 1
                        ]
                        cslice = corr_sb[:].rearrange("p (j h) -> p h j", j=2)[
                            :, i2 * 8 : i2 * 8 + 8, :
                        ]
                        nc.vector.tensor_add(edge, edge, cslice)
                        g = i2 // 4
                        j0 = (i2 % 4) * 2 * TILE
                        dst = stage[g * O : (g + 1) * O, j0 : j0 + 2 * TILE]
                        if i2 % 3 == 0:
                            nc.vector.tensor_copy(dst, ps[:])
                        else:
                            nc.scalar.copy(dst, ps[:])
                    for g in range(2):
                        nc.scalar.dma_start(
                            out_v[n, half, g], stage[g * O : (g + 1) * O, :]
                        )
                if n + 2 < IMGS:
                    load_image(n + 2)

    nc.compile()
    return nc


def _get_nc():
    if "nc" not in _CACHE:
        _CACHE["nc"] = _build()
    return _CACHE["nc"]


def _prep_weights(kernel: np.ndarray) -> np.ndarray:
    # reference reinterprets the raw (3,3,32,64) buffer as (C=32, kh, kw, O)
    w_eff = np.ascontiguousarray(kernel, dtype=np.float32).reshape(C, 3, 3, O)
    # wk[kw, kh*32+c, o] = w_eff[c, kh, kw, o]
    wk = np.ascontiguousarray(w_eff.transpose(2, 1, 0, 3).reshape(3, 96, O))
    return np.concatenate([wk, -wk[0:1], -wk[2:3]], axis=0)


def run(x: np.ndarray, kernel: np.ndarray, trace: bool = False):
    from concourse.bass_utils import run_bass_kernel_spmd

    if trace:
        try:
            from antenv.axon_hooks import (
                get_axon_ntff_profile_hook,
                set_axon_ntff_profile_hook,
            )
            from trn_agent_boot.trn_boot import _ntff_profile_via_ctypes

            if get_axon_ntff_profile_hook() is None:
                set_axon_ntff_profile_hook(
                    _ntff_profile_via_ctypes("/opt/axon/libaxon_pjrt.so")
                )
        except Exception:
            trace = False

    nc = _get_nc()
    x = np.ascontiguousarray(x, dtype=np.float32)
    wp = _prep_weights(kernel)
    in_maps = [
        {"x": x[i * IMGS : (i + 1) * IMGS], "w": wp} for i in range(N_CORES)
    ]
    res = run_bass_kernel_spmd(
        nc, in_maps, core_ids=list(range(N_CORES)), trace=trace
    )
    out = np.concatenate([res.results[i]["out"] for i in range(N_CORES)], axis=0)
    return out, res


def kernel(x: np.ndarray, kernel: np.ndarray) -> np.ndarray:
    out, _ = run(x, kernel)
    return out


if __name__ == "__main__":
    xs = np.random.randn(N_BATCH, C, H, W).astype(np.float32)
    ks = np.random.randn(3, 3, C, O).astype(np.float32)
    out = kernel(xs, ks)
    print(out.shape, out.dtype)
